# revision 1
# baseline (speedup 1.0000x reference)
"""FFT-based DCT-II on 8 trn2 NeuronCores (pipelined rev E).

Per core (256 rows): Makhoul DCT->real-FFT, four-step radix-64x64, twiddles
folded into stage-2 tables, conjugate symmetry (66 stage-1 slots incl. two
zero columns), mid-transpose via DRAM roundtrip. fp16 operands, fp32 psum.

Pipelining: x1 loaded in 4 chunks; stage-1 copybacks stream into 4 t_sb
tiles whose T-writes overlap stage 1; T2 read in m-chunks overlapping
stage 2; DMAs spread over sync/scalar/gpsimd queues.
"""

import numpy as np

N = 4096
R = 2048
RPC = 256

_state = {}


def _tables():
    n1 = np.arange(64)[:, None].astype(np.float64)
    j = np.arange(33)[None, :].astype(np.float64)
    F1c = np.cos(2 * np.pi * n1 * j / 64)
    F1s = -np.sin(2 * np.pi * n1 * j / 64)
    F1 = np.concatenate([F1c, F1s], axis=1)  # [64, 66]; cols 33 & 65 are 0
    f1_np = np.vstack([F1, F1]).astype(np.float16)  # [128, 66]

    n2v = np.arange(64)[:, None].astype(np.float64)
    k2v = np.arange(64)[None, :].astype(np.float64)

    def HH_single(k1):
        k = 64 * k2v + k1
        Gc = np.cos(2 * np.pi * n2v * k / N)
        Gs = -np.sin(2 * np.pi * n2v * k / N)
        cosE = np.cos(np.pi * k / (2 * N))
        sinE = np.sin(np.pi * k / (2 * N))
        sigma = 1.0 if k1 <= 32 else -1.0
        H1 = cosE * Gc + sinE * Gs
        H2 = sigma * (sinE * Gc - cosE * Gs)
        return np.concatenate([H1, H2], axis=0)  # [128, 64]

    HH = np.zeros((33, 128, 128))
    for a in range(1, 32):
        HH[a][:, :64] = HH_single(a)
        HH[a][:, 64:] = HH_single(64 - a)
    HH[0][:, :64] = HH_single(0)
    HH[32][:, 64:] = HH_single(32)
    # t2 partitions come from the (n c) DMA merge: p = 2*n2 + c
    rowperm = np.empty(128, dtype=np.int64)
    for n2 in range(64):
        for c in range(2):
            rowperm[2 * n2 + c] = c * 64 + n2
    HH = HH[:, rowperm, :]
    hh_np = HH.transpose(1, 0, 2).astype(np.float16).copy()  # [128, 33, 128]

    k1_arr = np.empty(64, dtype=np.int64)
    for a in range(32):
        k1_arr[2 * a] = a
        k1_arr[2 * a + 1] = (64 - a) if a > 0 else 32
    return f1_np, hh_np, k1_arr


def _t2_slice(t2_tiles, a):
    if a < 8:
        return t2_tiles[0][:, a, :]
    if a == 32:
        return t2_tiles[0][:, 8, :]
    j = 1 + (a - 8) // 8
    return t2_tiles[j][:, (a - 8) % 8, :]


def _build():
    import concourse.tile as tile
    from concourse import bacc, mybir

    f16 = mybir.dt.float16
    f32 = mybir.dt.float32

    nc = bacc.Bacc("TRN2", target_bir_lowering=False, debug=False, num_devices=8)
    x1_d = nc.dram_tensor("x1", [128, 8192], f16, kind="ExternalInput").ap()
    f1_d = nc.dram_tensor("f1", [128, 66], f16, kind="ExternalInput").ap()
    hh_d = nc.dram_tensor("hh", [128, 33, 128], f16, kind="ExternalInput").ap()
    y_d = nc.dram_tensor("y", [32, 2, 64, 256], f32, kind="ExternalOutput").ap()

    with tile.TileContext(nc) as tc:
        with (
            tc.tile_pool(name="const", bufs=1) as const,
            tc.tile_pool(name="data", bufs=1) as data,
            tc.tile_pool(name="dram", bufs=1, space="DRAM") as dram,
            tc.tile_pool(name="ps1", bufs=3, space="PSUM") as ps1,
            tc.tile_pool(name="ps2", bufs=2, space="PSUM") as ps2,
            tc.tile_pool(name="ysb", bufs=6) as ysb,
        ):
            f1_sb = const.tile([128, 66], f16)
            hh_sb = const.tile([128, 33, 128], f16)
            nc.sync.dma_start(f1_sb[:], f1_d)

            # x1 in 4 chunks (sync queue); hh deferred behind them
            x1_g = []
            for g in range(4):
                xg = data.tile([128, 2048], f16, name=f"x1_{g}")
                nc.sync.dma_start(xg[:], x1_d[:, 2048 * g : 2048 * g + 2048])
                x1_g.append(xg)
            nc.sync.dma_start(hh_sb[:], hh_d)

            # T in DRAM slot-major [s=(c,m), n2, r]: contiguous fast writes;
            # reads split by c across scalar/gpsimd queues (disjoint
            # partition halves -> disjoint DMA-engine sets, parallel).
            t_dram = dram.tile([64, 2, 33, 256], f16)  # [n2, c, m, r]
            t_sb_g = [
                data.tile([66, 16, 256], f16, name=f"tsb_{g}") for g in range(4)
            ]

            # stage 1: f in [0,16), psum tile per (p=f//2, h) holds 2 MMs.
            # Emit h-alternating so adjacent PE matmuls hit different row
            # groups and overlap in the array.
            cb = 0
            for p in range(8):
                tiles = [
                    ps1.tile([66, 2, 512], f32, name=f"s1ps_{p}_{h}", tag="s1ps")
                    for h in range(2)
                ]
                for j in range(2):
                    for h in range(2):
                        f = 2 * p + j
                        g, sl = f // 4, (f % 4) * 512
                        nc.tensor.matmul(
                            tiles[h][:, j, :],
                            f1_sb[64 * h : 64 * h + 64, :],
                            x1_g[g][64 * h : 64 * h + 64, sl : sl + 512],
                            start=True,
                            stop=True,
                        )
                for h in range(2):
                    dst = t_sb_g[p // 2][
                        :, (p % 2) * 8 : (p % 2) * 8 + 8, 128 * h : 128 * h + 128
                    ]
                    src = tiles[h][:].rearrange("s j (a b) -> s (j a) b", a=4)
                    if cb % 2 == 0:
                        nc.vector.tensor_copy(dst, src)
                    else:
                        nc.scalar.copy(dst, src)
                    cb += 1
                # write this n2 8-slice as soon as both h halves are done
                nc.sync.dma_start(
                    t_dram[8 * p : 8 * p + 8].rearrange("n c m r -> (c m) n r"),
                    t_sb_g[p // 2][:, (p % 2) * 8 : (p % 2) * 8 + 8, :],
                )

            # T2 read in m-chunks; per chunk: c=0 half on scalar queue,
            # c=1 half on gpsimd queue (parallel). Chunk0 carries m=32 too.
            t2_tiles = [
                data.tile([128, 9 if j == 0 else 8, 256], f16, name=f"t2_{j}")
                for j in range(4)
            ]

            # full-width reads via the (n c) partition merge
            t_rd = t_dram[:].rearrange("n c m r -> (n c) m r")
            nc.scalar.dma_start(t2_tiles[0][:, 0:8, :], t_rd[:, 0:8, :])
            nc.scalar.dma_start(t2_tiles[0][:, 8:9, :], t_rd[:, 32:33, :])
            for j in range(1, 4):
                nc.scalar.dma_start(
                    t2_tiles[j][:, 0:8, :], t_rd[:, 8 * j : 8 * j + 8, :]
                )

            # stage 2: 16 psum tiles, each two a's; a=0 accumulates m=0 and m=32
            for q in range(16):
                ps = ps2.tile([128, 512], f32)
                for i in range(2):
                    a = 2 * q + i
                    out = ps[:, 256 * i : 256 * i + 256]
                    if a == 0:
                        nc.tensor.matmul(
                            out, hh_sb[:, 0, :], _t2_slice(t2_tiles, 0),
                            start=True, stop=False,
                        )
                        nc.tensor.matmul(
                            out, hh_sb[:, 32, :], _t2_slice(t2_tiles, 32),
                            start=False, stop=True,
                        )
                    else:
                        nc.tensor.matmul(
                            out, hh_sb[:, a, :], _t2_slice(t2_tiles, a),
                            start=True, stop=True,
                        )
                y_sb = ysb.tile([128, 512], f32)
                if q % 2 == 0:
                    nc.vector.tensor_copy(y_sb[:], ps[:])
                else:
                    nc.scalar.copy(y_sb[:], ps[:])
                dst = y_d[2 * q : 2 * q + 2].rearrange("a d k r -> (d k) a r")
                src = y_sb[:].rearrange("p (a r) -> p a r", a=2)
                if q % 2 == 0:
                    nc.sync.dma_start(dst, src)
                else:
                    nc.scalar.dma_start(dst, src)

    nc.compile()
    return nc


def _pack_x1(x_rows):
    v = np.empty_like(x_rows)
    v[:, : N // 2] = x_rows[:, 0::2]
    v[:, N // 2 :] = x_rows[:, 1::2][:, ::-1]
    x1 = v.reshape(2, 128, 64, 64).transpose(0, 2, 3, 1).reshape(128, 8192)
    return np.ascontiguousarray(x1.astype(np.float16))


def kernel(x, _trace: bool = False):
    from concourse.bass_utils import run_bass_kernel_spmd

    x = np.asarray(x, dtype=np.float32)
    assert x.shape == (R, N)
    if "nc" not in _state:
        _state["nc"] = _build()
        _state["tables"] = _tables()
    nc = _state["nc"]
    f1_np, hh_np, k1_arr = _state["tables"]

    in_maps = []
    for c in range(8):
        in_maps.append(
            {
                "x1": _pack_x1(x[c * RPC : (c + 1) * RPC]),
                "f1": f1_np,
                "hh": hh_np,
            }
        )

    res = run_bass_kernel_spmd(nc, in_maps, list(range(8)), trace=_trace)

    y = np.empty((R, N), dtype=np.float32)
    for c in range(8):
        ydev = res.results[c]["y"]  # [32, 2, 64, 256]
        perm = ydev.transpose(3, 2, 0, 1).reshape(RPC, 64, 64)
        yc = np.empty((RPC, 64, 64), dtype=np.float32)
        yc[:, :, k1_arr] = perm
        y[c * RPC : (c + 1) * RPC] = yc.reshape(RPC, N)
    if _trace:
        _state["last_result"] = res
    return y



# revision 8
# speedup vs baseline: 1.0015x; 1.0015x over previous
"""FFT-based DCT-II on 8 trn2 NeuronCores (rev F).

Per core (256 rows): Makhoul DCT->real-FFT, four-step radix-64x64.
Stage 1 uses a block-diagonal [128,128] stationary (both row-halves in
one K=128 matmul, 64 dense output slots, single weight load) -- 16
matmuls instead of 32. Mid-transpose via DRAM roundtrip: slot-major
t_dram gives multi-KB write runs; strided reads merge (c,n2) onto
partitions. Stage 2: 33 matmuls vs per-a twiddle tables. y written
fp16 (cast to f32 on host).

Pipelining: x in 4 chunks + T-writes + c0-reads on sync queue; hh head
+ c1-reads + y on scalar; casts on vector+gpsimd; 5 dummy matmuls at
start lift the HAM clock gate before the first x chunk lands.
"""

import numpy as np

N = 4096
R = 2048
RPC = 256

_state = {}


def _tables():
    n1 = np.arange(64)[:, None].astype(np.float64)
    m = np.arange(33)[None, :].astype(np.float64)
    F1c = np.cos(2 * np.pi * n1 * m / 64)  # [64, 33]
    F1s = -np.sin(2 * np.pi * n1 * m / 64)  # [64, 33]
    # 64 dense slots: cos m=0..32, sin m=1..31
    F1p = np.concatenate([F1c, F1s[:, 1:32]], axis=1)  # [64, 64]
    w1 = np.zeros((128, 128))
    w1[:64, :64] = F1p
    w1[64:, 64:] = F1p
    w1_np = w1.astype(np.float16)

    n2v = np.arange(64)[:, None].astype(np.float64)
    k2v = np.arange(64)[None, :].astype(np.float64)

    def HH_single(k1):
        k = 64 * k2v + k1
        Gc = np.cos(2 * np.pi * n2v * k / N)
        Gs = -np.sin(2 * np.pi * n2v * k / N)
        cosE = np.cos(np.pi * k / (2 * N))
        sinE = np.sin(np.pi * k / (2 * N))
        sigma = 1.0 if k1 <= 32 else -1.0
        H1 = cosE * Gc + sinE * Gs
        H2 = sigma * (sinE * Gc - cosE * Gs)
        return np.concatenate([H1, H2], axis=0)  # [128, 64] rows = (c, n2)

    HH = np.zeros((33, 128, 128))
    for a in range(1, 32):
        HH[a][:, :64] = HH_single(a)
        HH[a][:, 64:] = HH_single(64 - a)
    HH[0][:, :64] = HH_single(0)
    HH[32][:, 64:] = HH_single(32)
    # sin(m=0)/sin(m=32) inputs are identically zero; zero their weight
    # rows so the memset t2 slots can't contribute.
    HH[0][64:, :] = 0.0
    HH[32][64:, :] = 0.0
    hh_np = HH.transpose(1, 0, 2).astype(np.float16).copy()  # [128, 33, 128]

    k1_arr = np.empty(64, dtype=np.int64)
    for a in range(32):
        k1_arr[2 * a] = a
        k1_arr[2 * a + 1] = (64 - a) if a > 0 else 32
    return w1_np, hh_np, k1_arr


def _build():
    import concourse.tile as tile
    from concourse import bacc, mybir

    f16 = mybir.dt.float16
    f32 = mybir.dt.float32

    nc = bacc.Bacc("TRN2", target_bir_lowering=False, debug=False, num_devices=8)
    x1_d = nc.dram_tensor("x1", [128, 8192], f16, kind="ExternalInput").ap()
    w1_d = nc.dram_tensor("w1", [128, 128], f16, kind="ExternalInput").ap()
    hh_d = nc.dram_tensor("hh", [128, 33, 128], f16, kind="ExternalInput").ap()
    y_d = nc.dram_tensor("y", [32, 2, 64, 256], f16, kind="ExternalOutput").ap()

    # transpose write groups: f-chunk ranges (each f covers 4 n2 values)
    GRP = [(0, 4), (4, 10), (10, 16)]

    with tile.TileContext(nc) as tc:
        with (
            tc.tile_pool(name="const", bufs=1) as const,
            tc.tile_pool(name="data", bufs=1) as data,
            tc.tile_pool(name="dram", bufs=1, space="DRAM") as dram,
            tc.tile_pool(name="ps1", bufs=3, space="PSUM") as ps1,
            tc.tile_pool(name="ps2", bufs=4, space="PSUM") as ps2,
            tc.tile_pool(name="ysb", bufs=4) as ysb,
        ):
            w1_sb = const.tile([128, 128], f16)
            hh_sb = const.tile([128, 33, 128], f16)
            x1_sb = data.tile([128, 8192], f16)
            t_sb = data.tile([128, 64, 128], f16)  # (h,slot) x n2 x w
            t2_sb = data.tile([128, 33, 2, 128], f16)  # (c,n2) x m x h x w
            t_dram = dram.tile([128, 64, 128], f16)  # (h,slot) x n2 x w
            warm_sb = data.tile([128, 512], f16)

            # prewarm scratch + zero the two t2 sin slots nothing writes
            nc.gpsimd.memset(warm_sb[:], 0.0)
            nc.gpsimd.memset(t2_sb[64:128, 0, :, :], 0.0)
            nc.gpsimd.memset(t2_sb[64:128, 32, :, :], 0.0)

            # input DMAs. sync: w1 + x chunks + hh tail. scalar: hh head.
            nc.sync.dma_start(w1_sb[:], w1_d)
            nc.scalar.dma_start(hh_sb[:, 0:8, :], hh_d[:, 0:8, :])
            nc.scalar.dma_start(hh_sb[:, 32:33, :], hh_d[:, 32:33, :])
            for g in range(4):
                sl = slice(2048 * g, 2048 * g + 2048)
                nc.sync.dma_start(x1_sb[:, sl], x1_d[:, sl])
            nc.sync.dma_start(hh_sb[:, 8:20, :], hh_d[:, 8:20, :])
            nc.sync.dma_start(hh_sb[:, 20:32, :], hh_d[:, 20:32, :])

            # prewarm: dummy matmuls lift the HAM clock gate early
            warm_ps = ps1.tile([128, 512], f32, name="warm", tag="s1ps")
            for _ in range(5):
                nc.tensor.matmul(
                    warm_ps[:], warm_sb[:, 0:128], warm_sb[:], start=True, stop=True
                )

            # stage 1: 16 K=128 matmuls, shared stationary w1
            gi = 0
            for f in range(16):
                ps = ps1.tile([128, 512], f32, name=f"s1_{f}", tag="s1ps")
                nc.tensor.matmul(
                    ps[:],
                    w1_sb[:],
                    x1_sb[:, 512 * f : 512 * f + 512],
                    start=True,
                    stop=True,
                )
                dst = t_sb[:, 4 * f : 4 * f + 4, :]
                src = ps[:].rearrange("p (n w) -> p n w", n=4)
                nc.vector.tensor_copy(dst, src)

                if gi < len(GRP) and f == GRP[gi][1] - 1:
                    fa, fb = GRP[gi]
                    n2a, n2b = 4 * fa, 4 * fb
                    nc.sync.dma_start(
                        t_dram[:, n2a:n2b, :], t_sb[:, n2a:n2b, :]
                    )
                    gi += 1

            # transpose reads: merge (c,n2) onto partitions, (m,h,w) free.
            # c=0: slots 0..32 -> m 0..32 ; c=1: slots 33..63 -> m 1..31
            def t_rd(h, sa, sb):
                return t_dram[64 * h + sa : 64 * h + sb, :, :].rearrange(
                    "s n w -> n s w"
                )

            # consumption order: m32 + m0:16 first (q0..3), then the rest
            for h in range(2):
                nc.sync.dma_start(t2_sb[0:64, 32:33, h, :], t_rd(h, 32, 33))
            for h in range(2):
                nc.sync.dma_start(t2_sb[0:64, 0:16, h, :], t_rd(h, 0, 16))
            for h in range(2):
                nc.scalar.dma_start(t2_sb[64:128, 1:16, h, :], t_rd(h, 33, 48))
            for h in range(2):
                nc.sync.dma_start(t2_sb[0:64, 16:32, h, :], t_rd(h, 16, 32))
            for h in range(2):
                nc.scalar.dma_start(t2_sb[64:128, 16:32, h, :], t_rd(h, 48, 64))

            # stage 2: 33 matmuls (a-pairs per psum tile), y out fp16
            def t2s(a):
                return t2_sb[:, a, :, :].rearrange("p h w -> p (h w)")

            ytile = None
            for q in range(16):
                ps = ps2.tile([128, 512], f32, name=f"s2_{q}", tag="s2ps")
                for i in range(2):
                    a = 2 * q + i
                    out = ps[:, 256 * i : 256 * i + 256]
                    if a == 0:
                        nc.tensor.matmul(
                            out, hh_sb[:, 0, :], t2s(0), start=True, stop=False
                        )
                        nc.tensor.matmul(
                            out, hh_sb[:, 32, :], t2s(32), start=False, stop=True
                        )
                    else:
                        nc.tensor.matmul(
                            out, hh_sb[:, a, :], t2s(a), start=True, stop=True
                        )
                j = q // 2
                if q % 2 == 0:
                    ytile = ysb.tile([128, 1024], f16, name=f"y_{j}", tag="ysb")
                dst = ytile[:, 512 * (q % 2) : 512 * (q % 2) + 512]
                if q % 2 == 0:
                    nc.scalar.copy(dst, ps[:])
                else:
                    nc.vector.tensor_copy(dst, ps[:])
                if q % 2 == 1:
                    ydst = y_d[4 * j : 4 * j + 4].rearrange("a d k r -> (d k) a r")
                    ysrc = ytile[:].rearrange("p (a r) -> p a r", a=4)
                    nc.gpsimd.dma_start(ydst, ysrc)

    nc.compile()
    return nc


def _pack_x1(x_rows):
    v = np.empty_like(x_rows)
    v[:, : N // 2] = x_rows[:, 0::2]
    v[:, N // 2 :] = x_rows[:, 1::2][:, ::-1]
    x1 = v.reshape(2, 128, 64, 64).transpose(0, 2, 3, 1).reshape(128, 8192)
    return np.ascontiguousarray(x1.astype(np.float16))


def kernel(x, _trace: bool = False):
    from concourse.bass_utils import run_bass_kernel_spmd

    x = np.asarray(x, dtype=np.float32)
    assert x.shape == (R, N)
    if "nc" not in _state:
        _state["nc"] = _build()
        _state["tables"] = _tables()
    nc = _state["nc"]
    w1_np, hh_np, k1_arr = _state["tables"]

    in_maps = []
    for c in range(8):
        in_maps.append(
            {
                "x1": _pack_x1(x[c * RPC : (c + 1) * RPC]),
                "w1": w1_np,
                "hh": hh_np,
            }
        )

    res = run_bass_kernel_spmd(nc, in_maps, list(range(8)), trace=_trace)

    y = np.empty((R, N), dtype=np.float32)
    for c in range(8):
        ydev = res.results[c]["y"].astype(np.float32)  # [32, 2, 64, 256]
        perm = ydev.transpose(3, 2, 0, 1).reshape(RPC, 64, 64)
        yc = np.empty((RPC, 64, 64), dtype=np.float32)
        yc[:, :, k1_arr] = perm
        y[c * RPC : (c + 1) * RPC] = yc.reshape(RPC, N)
    if _trace:
        _state["last_result"] = res
    return y


# revision 10
# speedup vs baseline: 1.0510x; 1.0495x over previous
"""FFT-based DCT-II on 8 trn2 NeuronCores (rev G, radix 128x32).

Per core (256 rows): Makhoul DCT->real-FFT, four-step radix-128x32.
Stage 1: 16 matmuls [K=128(n1), M=128 dense real-DFT slots, N=512],
one stationary, full-lane copies, rows NOT split across partitions
(w=256 stays in the free dim). T roundtrip through DRAM is clean on
both sides: writes are multi-KB runs, reads 512B runs. t2 carries the
(j, 64-j) pair layout (upper K-half holds reversed-m slots via
negative-stride DRAM reads) so stage 2 is 33 matmuls
[K=128, M=128, N=256] with block-diagonal pair weights. y fp16.
"""

import numpy as np

N = 4096
R = 2048
RPC = 256

_state = {}


def _tables():
    N1, N2 = 128, 32
    n1 = np.arange(N1)[:, None].astype(np.float64)
    jc = np.arange(65)[None, :].astype(np.float64)
    js = np.arange(1, 64)[None, :].astype(np.float64)
    F1c = np.cos(2 * np.pi * n1 * jc / N1)  # [128, 65]
    F1s = -np.sin(2 * np.pi * n1 * js / N1)  # [128, 63]
    w1_np = np.concatenate([F1c, F1s], axis=1).astype(np.float16)  # [128, 128]

    n2v = np.arange(N2)[:, None].astype(np.float64)
    k2v = np.arange(N2)[None, :].astype(np.float64)

    def HHs(k1):
        k = N1 * k2v + k1
        Gc = np.cos(2 * np.pi * n2v * k / N)
        Gs = -np.sin(2 * np.pi * n2v * k / N)
        cosE = np.cos(np.pi * k / (2 * N))
        sinE = np.sin(np.pi * k / (2 * N))
        sigma = 1.0 if k1 <= 64 else -1.0
        H1 = cosE * Gc + sinE * Gs
        H2 = sigma * (sinE * Gc - cosE * Gs)
        return np.concatenate([H1, H2], axis=0)  # [64, 32] rows (Bc n2, Bs n2)

    HH2 = np.zeros((33, 128, 128))
    for j in range(33):
        HH2[j][0:64, 0:32] = HHs(j)
        if 1 <= j <= 32:
            HH2[j][0:64, 32:64] = HHs(128 - j)
        if 0 <= j <= 31:
            HH2[j][64:128, 64:96] = HHs(64 - j)
        if 1 <= j <= 31:
            HH2[j][64:128, 96:128] = HHs(64 + j)
    # slots whose sin inputs are identically zero (and memset on device)
    HH2[0][32:64, :] = 0.0
    HH2[0][96:128, :] = 0.0
    hh_np = HH2.transpose(1, 0, 2).astype(np.float16).copy()  # [128, 33, 128]

    # output slot -> k1 map: psum partitions (g, h, k2)
    k1map = np.full((33, 2, 2), -1, dtype=np.int64)
    for j in range(33):
        k1map[j, 0, 0] = j
        if 1 <= j <= 32:
            k1map[j, 0, 1] = 128 - j
        if 0 <= j <= 31:
            k1map[j, 1, 0] = 64 - j
        if 1 <= j <= 31:
            k1map[j, 1, 1] = 64 + j
    slot_of_k1 = np.empty(128, dtype=np.int64)
    for j in range(33):
        for g in range(2):
            for h in range(2):
                k1 = k1map[j, g, h]
                if 0 <= k1 < 128:
                    slot_of_k1[k1] = j * 4 + g * 2 + h
    return w1_np, hh_np, slot_of_k1


def _build():
    import concourse.tile as tile
    from concourse import bacc, mybir

    f16 = mybir.dt.float16
    f32 = mybir.dt.float32

    nc = bacc.Bacc("TRN2", target_bir_lowering=False, debug=False, num_devices=8)
    x1_d = nc.dram_tensor("x1", [128, 8192], f16, kind="ExternalInput").ap()
    w1_d = nc.dram_tensor("w1", [128, 128], f16, kind="ExternalInput").ap()
    hh_d = nc.dram_tensor("hh", [128, 33, 128], f16, kind="ExternalInput").ap()
    y_d = nc.dram_tensor("y", [33, 2, 2, 32, 256], f16, kind="ExternalOutput").ap()

    # T write groups: f-chunk ranges (each f covers 2 n2 values)
    GRP = [(0, 6), (6, 12), (12, 16)]

    with tile.TileContext(nc) as tc:
        with (
            tc.tile_pool(name="const", bufs=1) as const,
            tc.tile_pool(name="data", bufs=1) as data,
            tc.tile_pool(name="dram", bufs=1, space="DRAM") as dram,
            tc.tile_pool(name="ps1", bufs=3, space="PSUM") as ps1,
            tc.tile_pool(name="ps2", bufs=3, space="PSUM") as ps2,
            tc.tile_pool(name="ysb", bufs=4) as ysb,
        ):
            w1_sb = const.tile([128, 128], f16)
            hh_sb = const.tile([128, 33, 128], f16)
            x1_sb = data.tile([128, 8192], f16)
            t_sb = data.tile([128, 32, 256], f16)  # slot x n2 x w
            t2_sb = data.tile([128, 33, 256], f16)  # (blk,c,n2) x j x w
            t_dram = dram.tile([128, 32, 256], f16)
            warm_sb = data.tile([128, 512], f16)

            nc.gpsimd.memset(warm_sb[:], 0.0)
            # zero sin slots with no source: lowBs j=0, upBs j=0
            nc.gpsimd.memset(t2_sb[32:64, 0, :], 0.0)
            nc.gpsimd.memset(t2_sb[96:128, 0, :], 0.0)

            # sync: x chunks then T writes then half the T reads.
            # scalar: w1 + hh (behind nothing; engines round-robin rings).
            for g in range(4):
                sl = slice(2048 * g, 2048 * g + 2048)
                nc.sync.dma_start(x1_sb[:, sl], x1_d[:, sl])
            nc.scalar.dma_start(w1_sb[:], w1_d)
            nc.scalar.dma_start(hh_sb[:, 0:8, :], hh_d[:, 0:8, :])
            nc.scalar.dma_start(hh_sb[:, 8:20, :], hh_d[:, 8:20, :])
            nc.scalar.dma_start(hh_sb[:, 20:33, :], hh_d[:, 20:33, :])

            # prewarm: lift the HAM clock gate before the first x lands
            warm_ps = ps1.tile([128, 512], f32, name="warm", tag="s1ps")
            for _ in range(5):
                nc.tensor.matmul(
                    warm_ps[:], warm_sb[:, 0:128], warm_sb[:], start=True, stop=True
                )

            # stage 1
            gi = 0
            for f in range(16):
                ps = ps1.tile([128, 512], f32, name=f"s1_{f}", tag="s1ps")
                nc.tensor.matmul(
                    ps[:],
                    w1_sb[:],
                    x1_sb[:, 512 * f : 512 * f + 512],
                    start=True,
                    stop=True,
                )
                dst = t_sb[:, 2 * f : 2 * f + 2, :]
                src = ps[:].rearrange("p (n w) -> p n w", n=2)
                if f % 2 == 0:
                    nc.vector.tensor_copy(dst, src)
                else:
                    nc.scalar.copy(dst, src)

                if gi < len(GRP) and f == GRP[gi][1] - 1:
                    fa, fb = GRP[gi]
                    n2a, n2b = 2 * fa, 2 * fb
                    nc.sync.dma_start(t_dram[:, n2a:n2b, :], t_sb[:, n2a:n2b, :])
                    gi += 1

            # T reads: dst partitions = n2, free = (j, w); src rows = slots.
            # lowBc: j<->s=j ; lowBs: s=64+j ; upBc: s=64-j ; upBs: s=128-j
            def rd(par0, ja, jb, srows):
                nc_eng = nc.sync if par0 in (0, 64) else nc.scalar
                nc_eng.dma_start(
                    t2_sb[par0 : par0 + 32, ja:jb, :],
                    t_dram[srows, :, :].rearrange("s n w -> n s w"),
                )

            rd(0, 0, 12, slice(0, 12))
            rd(32, 1, 12, slice(65, 76))
            rd(64, 0, 12, slice(64, 52, -1))
            rd(96, 1, 12, slice(127, 116, -1))
            rd(0, 12, 33, slice(12, 33))
            rd(32, 12, 33, slice(76, 97))
            rd(64, 12, 33, slice(52, 31, -1))
            rd(96, 12, 33, slice(116, 95, -1))

            # stage 2: 33 matmuls, pair weights; 17 psum tiles of <=2 j
            for q in range(17):
                nj = 2 if q < 16 else 1
                ps = ps2.tile([128, 512], f32, name=f"s2_{q}", tag="s2ps")
                for i in range(nj):
                    j = 2 * q + i
                    nc.tensor.matmul(
                        ps[:, 256 * i : 256 * i + 256],
                        hh_sb[:, j, :],
                        t2_sb[:, j, :],
                        start=True,
                        stop=True,
                    )
                ytile = ysb.tile([128, 512], f16, name=f"y_{q}", tag="ysb")
                cp_dst = ytile[:, 0 : 256 * nj]
                cp_src = ps[:, 0 : 256 * nj]
                if q % 2 == 0:
                    nc.vector.tensor_copy(cp_dst, cp_src)
                else:
                    nc.scalar.copy(cp_dst, cp_src)
                ydst = y_d[2 * q : 2 * q + nj].rearrange("j g h k w -> (g h k) j w")
                ysrc = ytile[:, 0 : 256 * nj].rearrange("p (j w) -> p j w", w=256)
                nc.gpsimd.dma_start(ydst, ysrc)

    nc.compile()
    return nc


def _pack_x1(x_rows):
    v = np.empty_like(x_rows)
    v[:, : N // 2] = x_rows[:, 0::2]
    v[:, N // 2 :] = x_rows[:, 1::2][:, ::-1]
    # x1[n1, n2, r] = v[r, 32*n1 + n2]
    x1 = v.reshape(RPC, 128, 32).transpose(1, 2, 0).reshape(128, 8192)
    return np.ascontiguousarray(x1.astype(np.float16))


def kernel(x, _trace: bool = False):
    from concourse.bass_utils import run_bass_kernel_spmd

    x = np.asarray(x, dtype=np.float32)
    assert x.shape == (R, N)
    if "nc" not in _state:
        _state["nc"] = _build()
        _state["tables"] = _tables()
    nc = _state["nc"]
    w1_np, hh_np, slot_of_k1 = _state["tables"]

    in_maps = []
    for c in range(8):
        in_maps.append(
            {
                "x1": _pack_x1(x[c * RPC : (c + 1) * RPC]),
                "w1": w1_np,
                "hh": hh_np,
            }
        )

    res = run_bass_kernel_spmd(nc, in_maps, list(range(8)), trace=_trace)

    y = np.empty((R, N), dtype=np.float32)
    for c in range(8):
        ydev = res.results[c]["y"].astype(np.float32)  # [33, 2, 2, 32, 256]
        # -> [w, k2, (j g h)]
        yk = ydev.transpose(4, 3, 0, 1, 2).reshape(RPC, 32, 132)
        yc = yk[:, :, slot_of_k1]  # [w, k2, k1]
        y[c * RPC : (c + 1) * RPC] = yc.reshape(RPC, N)
    if _trace:
        _state["last_result"] = res
    return y


# revision 11
# speedup vs baseline: 1.0970x; 1.0437x over previous
"""FFT-based DCT-II on 8 trn2 NeuronCores (rev H, radix 128x32).

Per core (256 rows): Makhoul DCT->real-FFT, four-step radix-128x32.
Stage 1: 16 matmuls [K=128(n1), M=128 dense real-DFT slots, N=512],
one stationary, full-lane psum->sbuf casts split vector/scalar, rows
kept in the free dim (w=256). Mid-transpose via DRAM roundtrip with
clean descriptors both ways (writes multi-KB runs, reads 512B runs);
the t2 pair layout (upper K-half reversed-m via negative-stride reads)
makes stage 2 exactly 33 matmuls [K=128, M=128, N=256]. y fp16.

Schedule: x in 5 chunks (small first) ahead of everything on sync; hh
head early / tail late on scalar; 4 T-write groups (small last); reads
in 3 j-chunks; y-writes as 9 big DMAs on sync (idle in stage 2).
"""

import numpy as np

N = 4096
R = 2048
RPC = 256

_state = {}


def _tables():
    N1, N2 = 128, 32
    n1 = np.arange(N1)[:, None].astype(np.float64)
    jc = np.arange(65)[None, :].astype(np.float64)
    js = np.arange(1, 64)[None, :].astype(np.float64)
    F1c = np.cos(2 * np.pi * n1 * jc / N1)  # [128, 65]
    F1s = -np.sin(2 * np.pi * n1 * js / N1)  # [128, 63]
    w1_np = np.concatenate([F1c, F1s], axis=1).astype(np.float16)  # [128, 128]

    n2v = np.arange(N2)[:, None].astype(np.float64)
    k2v = np.arange(N2)[None, :].astype(np.float64)

    def HHs(k1):
        k = N1 * k2v + k1
        Gc = np.cos(2 * np.pi * n2v * k / N)
        Gs = -np.sin(2 * np.pi * n2v * k / N)
        cosE = np.cos(np.pi * k / (2 * N))
        sinE = np.sin(np.pi * k / (2 * N))
        sigma = 1.0 if k1 <= 64 else -1.0
        H1 = cosE * Gc + sinE * Gs
        H2 = sigma * (sinE * Gc - cosE * Gs)
        return np.concatenate([H1, H2], axis=0)  # [64, 32] rows (Bc n2, Bs n2)

    HH2 = np.zeros((33, 128, 128))
    for j in range(33):
        HH2[j][0:64, 0:32] = HHs(j)
        if 1 <= j <= 32:
            HH2[j][0:64, 32:64] = HHs(128 - j)
        if 0 <= j <= 31:
            HH2[j][64:128, 64:96] = HHs(64 - j)
        if 1 <= j <= 31:
            HH2[j][64:128, 96:128] = HHs(64 + j)
    # slots whose sin inputs are identically zero (memset on device)
    HH2[0][32:64, :] = 0.0
    HH2[0][96:128, :] = 0.0
    hh_np = HH2.transpose(1, 0, 2).astype(np.float16).copy()  # [128, 33, 128]

    # output slot -> k1 map: psum partitions (g, h, k2)
    k1map = np.full((33, 2, 2), -1, dtype=np.int64)
    for j in range(33):
        k1map[j, 0, 0] = j
        if 1 <= j <= 32:
            k1map[j, 0, 1] = 128 - j
        if 0 <= j <= 31:
            k1map[j, 1, 0] = 64 - j
        if 1 <= j <= 31:
            k1map[j, 1, 1] = 64 + j
    slot_of_k1 = np.empty(128, dtype=np.int64)
    for j in range(33):
        for g in range(2):
            for h in range(2):
                k1 = k1map[j, g, h]
                if 0 <= k1 < 128:
                    slot_of_k1[k1] = j * 4 + g * 2 + h
    return w1_np, hh_np, slot_of_k1


def _build():
    import concourse.tile as tile
    from concourse import bacc, mybir

    f16 = mybir.dt.float16
    f32 = mybir.dt.float32

    nc = bacc.Bacc("TRN2", target_bir_lowering=False, debug=False, num_devices=8)
    x1_d = nc.dram_tensor("x1", [128, 8192], f16, kind="ExternalInput").ap()
    w1_d = nc.dram_tensor("w1", [128, 128], f16, kind="ExternalInput").ap()
    hh_d = nc.dram_tensor("hh", [128, 33, 128], f16, kind="ExternalInput").ap()
    y_d = nc.dram_tensor("y", [33, 2, 2, 32, 256], f16, kind="ExternalOutput").ap()

    XCH = [(0, 2), (2, 5), (5, 8), (8, 12), (12, 16)]  # x chunks in f units
    GRP = [(0, 5), (5, 9), (9, 13), (13, 16)]  # T write groups in f units
    RCH = [(0, 6), (6, 18), (18, 33)]  # read chunks in j units

    with tile.TileContext(nc) as tc:
        with (
            tc.tile_pool(name="const", bufs=1) as const,
            tc.tile_pool(name="data", bufs=1) as data,
            tc.tile_pool(name="dram", bufs=1, space="DRAM") as dram,
            tc.tile_pool(name="ps1", bufs=3, space="PSUM") as ps1,
            tc.tile_pool(name="ps2", bufs=3, space="PSUM") as ps2,
            tc.tile_pool(name="ysb", bufs=3) as ysb,
        ):
            w1_sb = const.tile([128, 128], f16)
            hh_sb = const.tile([128, 33, 128], f16)
            x1_sb = data.tile([128, 8192], f16)
            t_sb = data.tile([128, 32, 256], f16)  # slot x n2 x w
            t2_sb = data.tile([128, 33, 256], f16)  # (blk,c,n2) x j x w
            t_dram = dram.tile([128, 32, 256], f16)

            # zero sin slots with no source: lowBs j=0, upBs j=0
            nc.gpsimd.memset(t2_sb[32:64, 0, :], 0.0)
            nc.gpsimd.memset(t2_sb[96:128, 0, :], 0.0)

            # x chunks ahead of everything on sync; w1 + hh head on scalar
            for fa, fb in XCH:
                sl = slice(512 * fa, 512 * fb)
                nc.sync.dma_start(x1_sb[:, sl], x1_d[:, sl])
            nc.scalar.dma_start(w1_sb[:], w1_d)
            nc.scalar.dma_start(hh_sb[:, 0:6, :], hh_d[:, 0:6, :])

            # stage 1
            gi = 0
            for f in range(16):
                ps = ps1.tile([128, 512], f32, name=f"s1_{f}", tag="s1ps")
                nc.tensor.matmul(
                    ps[:],
                    w1_sb[:],
                    x1_sb[:, 512 * f : 512 * f + 512],
                    start=True,
                    stop=True,
                )
                dst = t_sb[:, 2 * f : 2 * f + 2, :]
                src = ps[:].rearrange("p (n w) -> p n w", n=2)
                if f % 2 == 0:
                    nc.vector.tensor_copy(dst, src)
                else:
                    nc.scalar.copy(dst, src)

                if gi < len(GRP) and f == GRP[gi][1] - 1:
                    fa, fb = GRP[gi]
                    n2a, n2b = 2 * fa, 2 * fb
                    nc.sync.dma_start(t_dram[:, n2a:n2b, :], t_sb[:, n2a:n2b, :])
                    gi += 1

            # hh tail: issued on scalar after its casts, lands before use
            nc.scalar.dma_start(hh_sb[:, 6:20, :], hh_d[:, 6:20, :])
            nc.scalar.dma_start(hh_sb[:, 20:33, :], hh_d[:, 20:33, :])

            # T reads: dst partitions = n2, free = (j, w); src rows = slots.
            # lowBc: s=j ; lowBs: s=64+j ; upBc: s=64-j ; upBs: s=128-j
            def rd(par0, ja, jb, srows, eng):
                eng.dma_start(
                    t2_sb[par0 : par0 + 32, ja:jb, :],
                    t_dram[srows, :, :].rearrange("s n w -> n s w"),
                )

            for ja, jb in RCH:
                rd(0, ja, jb, slice(ja, jb), nc.sync)
                rd(64, ja, jb, slice(64 - ja, 64 - jb, -1), nc.sync)
                ja1 = max(ja, 1)
                rd(32, ja1, jb, slice(64 + ja1, 64 + jb), nc.scalar)
                rd(96, ja1, jb, slice(128 - ja1, 128 - jb, -1), nc.scalar)

            # stage 2: 33 matmuls, pair weights; 17 psum tiles of <=2 j;
            # y staged in [128,1024] tiles (4 j) written on sync
            ytile = None
            for q in range(17):
                nj = 2 if q < 16 else 1
                ps = ps2.tile([128, 512], f32, name=f"s2_{q}", tag="s2ps")
                for i in range(nj):
                    j = 2 * q + i
                    nc.tensor.matmul(
                        ps[:, 256 * i : 256 * i + 256],
                        hh_sb[:, j, :],
                        t2_sb[:, j, :],
                        start=True,
                        stop=True,
                    )
                if q % 2 == 0:
                    ytile = ysb.tile([128, 1024], f16, name=f"y_{q//2}", tag="ysb")
                cp_dst = ytile[:, 512 * (q % 2) : 512 * (q % 2) + 256 * nj]
                cp_src = ps[:, 0 : 256 * nj]
                if q % 2 == 0:
                    nc.vector.tensor_copy(cp_dst, cp_src)
                else:
                    nc.scalar.copy(cp_dst, cp_src)
                if q % 2 == 1 or q == 16:
                    j0 = 4 * (q // 2)
                    njj = 4 if q % 2 == 1 else 1
                    ydst = y_d[j0 : j0 + njj].rearrange("j g h k w -> (g h k) j w")
                    ysrc = ytile[:, 0 : 256 * njj].rearrange(
                        "p (j w) -> p j w", w=256
                    )
                    nc.sync.dma_start(ydst, ysrc)

    nc.compile()
    return nc


def _pack_x1(x_rows):
    v = np.empty_like(x_rows)
    v[:, : N // 2] = x_rows[:, 0::2]
    v[:, N // 2 :] = x_rows[:, 1::2][:, ::-1]
    # x1[n1, n2, r] = v[r, 32*n1 + n2]
    x1 = v.reshape(RPC, 128, 32).transpose(1, 2, 0).reshape(128, 8192)
    return np.ascontiguousarray(x1.astype(np.float16))


def kernel(x, _trace: bool = False):
    from concourse.bass_utils import run_bass_kernel_spmd

    x = np.asarray(x, dtype=np.float32)
    assert x.shape == (R, N)
    if "nc" not in _state:
        _state["nc"] = _build()
        _state["tables"] = _tables()
    nc = _state["nc"]
    w1_np, hh_np, slot_of_k1 = _state["tables"]

    in_maps = []
    for c in range(8):
        in_maps.append(
            {
                "x1": _pack_x1(x[c * RPC : (c + 1) * RPC]),
                "w1": w1_np,
                "hh": hh_np,
            }
        )

    res = run_bass_kernel_spmd(nc, in_maps, list(range(8)), trace=_trace)

    y = np.empty((R, N), dtype=np.float32)
    for c in range(8):
        ydev = res.results[c]["y"].astype(np.float32)  # [33, 2, 2, 32, 256]
        yk = ydev.transpose(4, 3, 0, 1, 2).reshape(RPC, 32, 132)
        y[c * RPC : (c + 1) * RPC] = yk[:, :, slot_of_k1].reshape(RPC, N)
    if _trace:
        _state["last_result"] = res
    return y
